# revision 15
# baseline (speedup 1.0000x reference)
"""GQA attention (B=1, S=2048, H=2048, 32 q-heads / 8 kv-heads, hd=64)
on 8 Trainium2 NeuronCores.

Sharding: tensor-parallel over heads for QKV+attention (core c owns
q-heads 4c..4c+3 and kv-head c), then sequence-parallel o_proj: per
1024-query round, two AllToAlls (one per head-pair) redistribute the
transposed, normalized attention output so core j owns query rows
{128j..128j+128, 1024+128j..+128}; each core holds the FULL wo (bf16)
and computes its 256 output rows locally. Host concatenates.

All large matmuls are bf16 (fp32r lowers to fp32_mode=HIGH at ~3
cycles/row on HW; bf16 runs at 1). Engine queues are FIFO in emission
order, so the program is emitted interleaved:
  warmup-MMs, A0, B0, A1, B1(kv), qc0, B1(q0), qc1(+round-0 A2As),
  B1(q1), qc2, round-0 o_proj, qc3(+round-1 A2As), round-1 o_proj
q/k/v and V_aug are split into per-1024-column-half tiles so qc0/qc1
depend only on half 0. Big const DMAs (wo, cos/sin) go on the GpSimd
queue; xT tiles alternate sync/scalar queues; a small AllToAll early
prewarms the collective path. Normalize+staging run per (q-chunk,
head-pair) so the tail only waits on the last chunk's chain.
"""
import numpy as np
import sys

sys.path.insert(0, "/opt/trn_rl_repo")

import concourse.bacc as bacc  # noqa: E402
import concourse.mybir as mybir  # noqa: E402
import concourse.tile as tile  # noqa: E402
from concourse import bass_utils  # noqa: E402

f32 = mybir.dt.float32
bf16 = mybir.dt.bfloat16
AF = mybir.ActivationFunctionType
BF16NP = mybir.dt.np(bf16)

N_CORES = 8
S = 2048
HID = 2048
HD = 64
ROPE_THETA = 10000.0
RMS_EPS = 1e-6
SCALING = HD ** -0.5              # 0.125
NK = HID // 128                   # 16 contraction tiles
NQC = S // 512                    # 4 q chunks
NKT = S // 128                    # 16 kpos tiles

_NC_CACHE = None
LAST_RESULTS = None


def _build():
    nc = bacc.Bacc("TRN2", target_bir_lowering=False, debug=False,
                   num_devices=N_CORES)

    def din(name, shape, dt):
        return nc.dram_tensor(name, shape, dt, kind="ExternalInput").ap()

    xT = din("xT", [HID, S], bf16)
    # host-pretiled: row p, col block t = original rows 128t+p
    wq0 = din("wq0", [128, HID], bf16)
    wq1 = din("wq1", [128, HID], bf16)
    wkv = din("wkv", [128, HID], bf16)     # [wv | wk] columns pretiled
    wof = din("wof", [128, NK * HID], bf16)  # FULL wo, pretiled
    cos2 = din("cos2", [128, S], bf16)
    ss2 = din("ss2", [128, S], bf16)
    ew_q = din("ew_q", [2, 128], bf16)
    ew_k = din("ew_k", [2, 128], bf16)
    e2 = din("e2", [2, 128], bf16)
    e2t = din("e2t", [128, 2], bf16)
    mask = din("mask", [128, 128], bf16)
    ident = din("ident", [64, 64], bf16)

    out_rs = nc.dram_tensor("out_rs", [256, S], f32,
                            kind="ExternalOutput").ap()

    with tile.TileContext(nc) as tc:
        with tc.tile_pool(name="consts", bufs=1) as cp, \
             tc.tile_pool(name="dram", bufs=1, space="DRAM") as dp:
            c_wq0 = cp.tile([128, HID], bf16, tag="w")
            c_wq1 = cp.tile([128, HID], bf16, tag="w2")
            c_wkv = cp.tile([128, HID], bf16, tag="w3")
            c_wo = cp.tile([128, NK * HID], bf16, tag="w4")
            c_cos = cp.tile([128, S], bf16, tag="c1")
            c_ss = cp.tile([128, S], bf16, tag="c2")
            c_ewq = cp.tile([2, 128], bf16, tag="c3")
            c_ewk = cp.tile([2, 128], bf16, tag="c4")
            c_e2 = cp.tile([2, 128], bf16, tag="c5")
            c_e2t = cp.tile([128, 2], bf16, tag="c5t")
            c_mask = cp.tile([128, 128], bf16, tag="c6")
            c_id = cp.tile([64, 64], bf16, tag="c7")
            c_eps = cp.tile([2, 1], f32, tag="c8")
            c_scr = cp.tile([128, 640], bf16, tag="c9")

            # PE warmup: memset scratch, then dummy matmuls so the HAM
            # clock gate is at 8/8 when the first real matmul lands.
            nc.vector.memset(c_scr[:], 0.0)
            nc.vector.memset(c_eps[:], RMS_EPS)

            # phase-A weights + first xT tiles on sync (gate first MMs)
            nc.sync.dma_start(c_wq0[:], wq0)
            nc.sync.dma_start(c_wq1[:], wq1)
            nc.sync.dma_start(c_wkv[:], wkv)
            # small phase-B consts on the scalar queue
            nc.scalar.dma_start(c_e2t[:], e2t)
            nc.scalar.dma_start(c_ewq[:], ew_q)
            nc.scalar.dma_start(c_ewk[:], ew_k)
            nc.scalar.dma_start(c_id[:], ident)
            nc.scalar.dma_start(c_e2[:], e2)
            # rope tables + CDE consts + full wo on the gpsimd queue
            nc.gpsimd.dma_start(c_cos[:], cos2)
            nc.gpsimd.dma_start(c_ss[:], ss2)
            nc.gpsimd.dma_start(c_mask[:], mask)
            for h in range(8):
                cs_ = slice(4096 * h, 4096 * (h + 1))
                nc.gpsimd.dma_start(c_wo[:, cs_], wof[:, cs_])

            qkv = {
                "q0": cp.tile([128, S], bf16, tag="q0", name="q0"),
                "q1": cp.tile([128, S], bf16, tag="q1", name="q1"),
                "kv": cp.tile([128, S], bf16, tag="kv", name="kv"),
            }
            # per-half rope'd q/k and V_aug tiles
            qr0h = [cp.tile([128, 1024], bf16, tag=f"qr0{h}",
                            name=f"qr0{h}") for h in range(2)]
            qr1h = [cp.tile([128, 1024], bf16, tag=f"qr1{h}",
                            name=f"qr1{h}") for h in range(2)]
            krdh = [cp.tile([128, 1024], bf16, tag=f"krd{h}",
                            name=f"krd{h}") for h in range(2)]
            vah = [cp.tile([128, 8 * (HD + 1)], bf16, tag=f"va{h}",
                           name=f"va{h}") for h in range(2)]

            attn_bf = [cp.tile([128, S], bf16, tag=f"abf{i}",
                               name=f"abf{i}") for i in range(2)]
            l_sb = [cp.tile([2, S], bf16, tag=f"lsb{i}", name=f"lsb{i}")
                    for i in range(2)]

            a2a_in = [[dp.tile([1024, 128], bf16, name=f"a2ai{r}{h}")
                       for h in range(2)] for r in range(2)]
            a2a_out = [[dp.tile([1024, 128], bf16, name=f"a2ao{r}{h}")
                        for h in range(2)] for r in range(2)]
            pw_in = dp.tile([8, 16], bf16, name="pwi")
            pw_out = dp.tile([8, 16], bf16, name="pwo")

            # collective prewarm (tiny AllToAll — pays the ncfw
            # first-call cost long before round 0 needs it)
            pws = cp.tile([8, 16], bf16, tag="pw")
            nc.vector.memset(pws[:], 0.0)
            nc.gpsimd.dma_start(pw_in[:, :], pws[:])
            nc.gpsimd.collective_compute(
                "AllToAll", mybir.AluOpType.bypass,
                replica_groups=[list(range(N_CORES))],
                ins=[pw_in[:, :].opt()], outs=[pw_out[:, :].opt()])

            # ================ phase A+B, halves ================
            with tc.tile_pool(name="xt", bufs=3) as xp, \
                 tc.tile_pool(name="sbB", bufs=2) as sbB:

                def phase_a(qh, psA, psM):
                    hs = slice(1024 * qh, 1024 * qh + 1024)
                    pq = [psA.tile([128, 1024], f32, tag="pa",
                                   name=f"pa{qh}_{j}") for j in range(3)]
                    for t in range(NK):
                        xt = xp.tile([128, 1024], bf16, tag="xt")
                        eng = nc.sync if t % 2 == 0 else nc.scalar
                        eng.dma_start(xt[:],
                                      xT[128 * t:128 * (t + 1), hs])
                        st = (t == 0)
                        sp = (t == NK - 1)
                        tc_ = slice(128 * t, 128 * (t + 1))
                        for j, w in ((0, c_wq0), (1, c_wq1), (2, c_wkv)):
                            nc.tensor.matmul(pq[j][:, 0:512], w[:, tc_],
                                             xt[:, 0:512],
                                             start=st, stop=sp)
                            nc.tensor.matmul(pq[j][:, 512:1024],
                                             w[:, tc_], xt[:, 512:1024],
                                             start=st, stop=sp)
                    for j, key in ((0, "q0"), (1, "q1"), (2, "kv")):
                        nc.vector.tensor_copy(qkv[key][:, hs], pq[j][:])

                def phase_b_spec(qh, si, key, ew, dst, is_kv, psM,
                                 ptag="m"):
                    hs = slice(1024 * qh, 1024 * qh + 1024)
                    src = qkv[key]
                    if is_kv:
                        nc.gpsimd.memset(vah[qh][:], 1.0)
                        for lt in range(8):
                            ptr = psM.tile([128, 64], bf16, tag=ptag,
                                           name=f"ptr{qh}_{lt}")
                            nc.tensor.transpose(
                                ptr[:],
                                src[0:64, 1024 * qh + 128 * lt:
                                    1024 * qh + 128 * (lt + 1)],
                                c_id[:])
                            nc.vector.tensor_copy(
                                vah[qh][:, (HD + 1) * lt:
                                        (HD + 1) * lt + HD],
                                ptr[:])
                    sq = sbB.tile([128, 1024], bf16, tag="sq",
                                  bufs=2, name=f"sq{qh}_{si}")
                    nc.vector.tensor_mul(sq[:], src[:, hs], src[:, hs])
                    rstds = {}
                    for u in range(2):
                        us = slice(512 * u, 512 * u + 512)
                        pss = psM.tile([2, 512], f32, tag=ptag,
                                       name=f"ss{qh}_{si}_{u}")
                        nc.tensor.matmul(pss[:], c_e2t[:], sq[:, us],
                                         start=True, stop=True)
                        lnv = sbB.tile([2, 512], bf16, tag="lnv",
                                       bufs=4, name=f"lnv{qh}{si}{u}")
                        nc.scalar.activation(lnv[:], pss[:], AF.Ln,
                                             scale=1.0 / HD,
                                             bias=c_eps[:])
                        rr = sbB.tile([2, 512], bf16, tag="rstdr",
                                      bufs=4, name=f"rr{qh}{si}{u}")
                        nc.scalar.activation(rr[:], lnv[:],
                                             AF.Exp, scale=-0.5)
                        rstds[u] = rr
                    rows = slice(64, 128) if is_kv else slice(0, 128)
                    nrm = sbB.tile([128, 1024], f32, tag="nrm",
                                   bufs=2, name=f"nrm{qh}_{si}")
                    for u in range(2):
                        cs = slice(1024 * qh + 512 * u,
                                   1024 * qh + 512 * u + 512)
                        us = slice(512 * u, 512 * u + 512)
                        pb = psM.tile([128, 512], f32, tag=ptag,
                                      name=f"pb{qh}_{si}_{u}")
                        nc.tensor.matmul(pb[:], ew[:], rstds[u][:],
                                         start=True, stop=True)
                        nc.vector.tensor_mul(nrm[rows, us],
                                             src[rows, cs], pb[rows, :])
                    # rope
                    sh = sbB.tile([128, 1024], f32, tag="sh",
                                  bufs=2, name=f"sh{qh}_{si}")
                    if is_kv:
                        nc.sync.dma_start(sh[64:96, :], nrm[96:128, :])
                        nc.sync.dma_start(sh[96:128, :], nrm[64:96, :])
                    else:
                        nc.sync.dma_start(sh[0:32, :], nrm[32:64, :])
                        nc.sync.dma_start(sh[32:64, :], nrm[0:32, :])
                        nc.sync.dma_start(sh[64:96, :], nrm[96:128, :])
                        nc.sync.dma_start(sh[96:128, :], nrm[64:96, :])
                    t2 = sbB.tile([128, 1024], f32, tag="t2",
                                  bufs=1, name=f"t2{qh}_{si}")
                    nc.vector.tensor_mul(t2[rows, :], sh[rows, :],
                                         c_ss[rows, hs])
                    t1 = sbB.tile([128, 1024], f32, tag="sh",
                                  bufs=2, name=f"t1{qh}_{si}")
                    nc.vector.tensor_mul(t1[rows, :], nrm[rows, :],
                                         c_cos[rows, hs])
                    nc.vector.tensor_add(dst[rows, :], t1[rows, :],
                                         t2[rows, :])
                    if is_kv:
                        nc.sync.dma_start(dst[0:64, :], dst[64:128, :])

                # ---- scope 1: warmup + A0 + B0 + A1 ----
                with tc.tile_pool(name="psA", bufs=3,
                                  space="PSUM") as psA, \
                     tc.tile_pool(name="psM", bufs=2,
                                  space="PSUM") as psM:
                    pwm = psM.tile([128, 512], f32, tag="m", name="pwm")
                    for i in range(32):
                        nc.tensor.matmul(pwm[:], c_scr[:, 0:128],
                                         c_scr[:, 128:640],
                                         start=True, stop=True)
                    phase_a(0, psA, psM)
                    for si, (key, ew, dst, ik) in enumerate((
                            ("kv", c_ewk, krdh[0], True),
                            ("q0", c_ewq, qr0h[0], False),
                            ("q1", c_ewq, qr1h[0], False))):
                        phase_b_spec(0, si, key, ew, dst, ik, psM)
                    phase_a(1, psA, psM)

                # ---- scope 2: B1 interleaved with CDE ----
                with tc.tile_pool(name="sbC", bufs=4) as sbC, \
                     tc.tile_pool(name="atk", bufs=2) as akp, \
                     tc.tile_pool(name="psS", bufs=2,
                                  space="PSUM") as psS, \
                     tc.tile_pool(name="psPV", bufs=2,
                                  space="PSUM") as psPV, \
                     tc.tile_pool(name="psO", bufs=2,
                                  space="PSUM") as psO:

                    def qchunk(qc):
                        qs = slice(512 * qc, 512 * qc + 512)
                        qhh = qc // 2
                        for hp, qrh in ((0, qr0h), (1, qr1h)):
                            qr = qrh[qhh]
                            ppv_a = psPV.tile([65, 512], f32, tag="pv")
                            ppv_b = psPV.tile([65, 512], f32, tag="pv")
                            ntile = 4 * qc + 4
                            for t in range(ntile):
                                r = t - 4 * qc
                                off = max(0, r) * 128
                                qlo = 512 * qc + off - 1024 * qhh
                                qlen = 512 * qc + 512 - 1024 * qhh - qlo
                                kh = t // 8
                                krd = krdh[kh]
                                v_aug = vah[kh]
                                tl = t - 8 * kh
                                kc = slice(128 * tl, 128 * (tl + 1))
                                vs = slice((HD + 1) * tl,
                                           (HD + 1) * tl + HD + 1)
                                st = (t == 0)
                                sp = (t == ntile - 1)
                                ps_s = psS.tile([128, 1024], f32,
                                                tag="s")
                                nc.tensor.matmul(
                                    ps_s[:, 0:qlen], krd[0:64, kc],
                                    qr[0:64, qlo:qlo + qlen],
                                    start=True, stop=True)
                                nc.tensor.matmul(
                                    ps_s[:, 512:512 + qlen],
                                    krd[64:128, kc],
                                    qr[64:128, qlo:qlo + qlen],
                                    start=True, stop=True)
                                pt = sbC.tile([128, 1024], bf16,
                                              tag="pt")
                                if r >= 0:
                                    nc.scalar.activation(
                                        pt[:, 0:512 + qlen],
                                        ps_s[:, 0:512 + qlen],
                                        AF.Exp, scale=SCALING)
                                    nc.vector.tensor_mul(
                                        pt[:, 0:128], pt[:, 0:128],
                                        c_mask[:])
                                    nc.vector.tensor_mul(
                                        pt[:, 512:640], pt[:, 512:640],
                                        c_mask[:])
                                else:
                                    nc.scalar.activation(
                                        pt[:, 0:1024], ps_s[:, 0:1024],
                                        AF.Exp, scale=SCALING)
                                nc.tensor.matmul(
                                    ppv_a[:, off:512], v_aug[:, vs],
                                    pt[:, 0:qlen], start=st, stop=sp)
                                nc.tensor.matmul(
                                    ppv_b[:, off:512], v_aug[:, vs],
                                    pt[:, 512:512 + qlen],
                                    start=st, stop=sp)
                            # per-hp: stage, 1/l, normalize, a2a-stage
                            for half, ppv in ((0, ppv_a), (1, ppv_b)):
                                stg = sbC.tile([65, 512], bf16,
                                               tag="stg", bufs=3)
                                nc.vector.tensor_copy(stg[:], ppv[:])
                                nc.sync.dma_start(
                                    attn_bf[hp][64 * half:
                                                64 * half + 64, qs],
                                    stg[0:64, :])
                                nc.sync.dma_start(
                                    l_sb[hp][half:half + 1, qs],
                                    stg[64:65, :])
                            rl = sbC.tile([2, 512], f32, tag="lnl",
                                          bufs=2, name=f"rl{qc}_{hp}")
                            nc.vector.reciprocal(rl[:],
                                                 l_sb[hp][:, qs])
                            rlb = sbC.tile([2, 512], bf16, tag="rlb",
                                           bufs=2, name=f"rb{qc}_{hp}")
                            nc.vector.tensor_copy(rlb[:], rl[:])
                            pb = psO.tile([128, 512], f32, tag="o")
                            nc.tensor.matmul(pb[:], c_e2[:], rlb[:],
                                             start=True, stop=True)
                            nc.vector.tensor_mul(
                                attn_bf[hp][:, qs],
                                attn_bf[hp][:, qs], pb[:])
                            rnd = qc // 2
                            for j in range(4 * (qc % 2),
                                           4 * (qc % 2) + 4):
                                qq = 1024 * rnd + 128 * j
                                nc.gpsimd.dma_start(
                                    a2a_in[rnd][hp][128 * j:
                                                    128 * j + 128, :],
                                    attn_bf[hp][:, qq:qq + 128])
                            if qc % 2 == 1:
                                nc.gpsimd.collective_compute(
                                    "AllToAll",
                                    mybir.AluOpType.bypass,
                                    replica_groups=[
                                        list(range(N_CORES))],
                                    ins=[a2a_in[rnd][hp][:, :].opt()],
                                    outs=[a2a_out[rnd][hp][:, :].opt()],
                                )

                    def oproj(rnd):
                        attk = akp.tile([128, S], bf16, tag="atk")
                        for hp in range(2):
                            for c in range(N_CORES):
                                kk = 2 * c + hp
                                nc.gpsimd.dma_start(
                                    attk[:, 128 * kk:128 * (kk + 1)],
                                    a2a_out[rnd][hp][128 * c:
                                                     128 * (c + 1), :])
                        pos = [psO.tile([128, 512], f32, tag="o",
                                        name=f"po{rnd}_0"),
                               psO.tile([128, 512], f32, tag="o",
                                        name=f"po{rnd}_1"),
                               psS.tile([128, 512], f32, tag="s",
                                        name=f"po{rnd}_2"),
                               psS.tile([128, 512], f32, tag="s",
                                        name=f"po{rnd}_3")]
                        for hp in range(2):
                            for c in range(N_CORES):
                                kk = 2 * c + hp
                                for n in range(4):
                                    nc.tensor.matmul(
                                        pos[n][:],
                                        attk[:, 128 * kk:
                                             128 * (kk + 1)],
                                        c_wo[:, HID * kk + 512 * n:
                                             HID * kk + 512 * n + 512],
                                        start=(hp == 0 and c == 0),
                                        stop=(hp == 1 and
                                              c == N_CORES - 1))
                        for n in range(4):
                            ost = sbC.tile([128, 512], f32, tag="ost",
                                           bufs=2)
                            nc.vector.tensor_copy(ost[:], pos[n][:])
                            nc.gpsimd.dma_start(
                                out_rs[128 * rnd:128 * rnd + 128,
                                       512 * n:512 * n + 512],
                                ost[:])

                    phase_b_spec(1, 0, "kv", c_ewk, krdh[1], True,
                                 psO, ptag="o")
                    qchunk(0)
                    phase_b_spec(1, 1, "q0", c_ewq, qr0h[1], False,
                                 psO, ptag="o")
                    qchunk(1)
                    phase_b_spec(1, 2, "q1", c_ewq, qr1h[1], False,
                                 psO, ptag="o")
                    qchunk(2)
                    oproj(0)
                    qchunk(3)
                    oproj(1)

    nc.compile()
    return nc


def _host_prep(hidden_states, position_ids, wq, wk, wv, wo, q_ln_w, k_ln_w):
    x = np.asarray(hidden_states, dtype=np.float32)[0]        # [S, HID]
    xT = np.ascontiguousarray(x.T).astype(BF16NP)             # [HID, S]
    pos = np.asarray(position_ids)[0].astype(np.float32)      # [S]
    inv = 1.0 / (ROPE_THETA ** (np.arange(0, HD, 2, dtype=np.float32) / HD))
    ang = pos[:, None] * inv[None, :]                         # [S, 32]
    emb = np.concatenate([ang, ang], axis=1)                  # [S, 64]
    cosT = np.cos(emb).T.astype(np.float32)                   # [64, S]
    sinT = np.sin(emb).T.astype(np.float32)
    ss = sinT.copy()
    ss[0:32] = -sinT[0:32]
    cos2 = np.tile(cosT, (2, 1))
    ss2 = np.tile(ss, (2, 1))

    e2 = np.zeros((2, 128), dtype=np.float32)
    e2[0, 0:64] = 1.0
    e2[1, 64:128] = 1.0
    ew_q = np.zeros((2, 128), dtype=np.float32)
    ew_q[0, 0:64] = q_ln_w
    ew_q[1, 64:128] = q_ln_w
    ew_k = np.zeros((2, 128), dtype=np.float32)
    ew_k[1, 64:128] = k_ln_w
    msk = (np.arange(128)[:, None] <= np.arange(128)[None, :]) \
        .astype(np.float32)
    ident = np.eye(64, dtype=np.float32)

    wq_ = np.asarray(wq, dtype=np.float32)
    wk_ = np.asarray(wk, dtype=np.float32)
    wv_ = np.asarray(wv, dtype=np.float32)
    wo_ = np.asarray(wo, dtype=np.float32)

    def pretile(w):  # [HID, N] -> [128, NK*N] ktile-blocked
        n = w.shape[1]
        return np.ascontiguousarray(
            w.reshape(NK, 128, n).transpose(1, 0, 2).reshape(128, NK * n))

    wof = pretile(wo_).astype(BF16NP)

    in_maps = []
    for c in range(N_CORES):
        qcols = slice(256 * c, 256 * (c + 1))
        kvcols = slice(64 * c, 64 * (c + 1))
        wq_c = np.ascontiguousarray(wq_[:, qcols])
        wkv_c = np.concatenate([wv_[:, kvcols], wk_[:, kvcols]], axis=1)
        in_maps.append({
            "xT": xT,
            "wq0": pretile(wq_c[:, 0:128]).astype(BF16NP),
            "wq1": pretile(wq_c[:, 128:256]).astype(BF16NP),
            "wkv": pretile(wkv_c).astype(BF16NP),
            "wof": wof,
            "cos2": cos2.astype(BF16NP),
            "ss2": ss2.astype(BF16NP),
            "ew_q": ew_q.astype(BF16NP),
            "ew_k": ew_k.astype(BF16NP),
            "e2": e2.astype(BF16NP),
            "e2t": np.ascontiguousarray(e2.T).astype(BF16NP),
            "mask": msk.astype(BF16NP),
            "ident": ident.astype(BF16NP),
        })
    return in_maps


def kernel(hidden_states, position_ids, wq, wk, wv, wo, q_ln_w, k_ln_w):
    global _NC_CACHE, LAST_RESULTS
    if _NC_CACHE is None:
        _NC_CACHE = _build()
    nc = _NC_CACHE
    in_maps = _host_prep(hidden_states, position_ids, wq, wk, wv, wo,
                         q_ln_w, k_ln_w)
    res = bass_utils.run_bass_kernel_spmd(
        nc, in_maps, core_ids=list(range(N_CORES)))
    LAST_RESULTS = res
    out = np.empty((S, HID), dtype=np.float32)
    for c in range(N_CORES):
        o_c = res.results[c]["out_rs"]        # [256, 2048]
        for rnd in range(2):
            out[1024 * rnd + 128 * c:1024 * rnd + 128 * c + 128, :] = \
                o_c[128 * rnd:128 * rnd + 128, :]
    return out.reshape(1, S, HID)


# revision 17
# speedup vs baseline: 1.1432x; 1.1432x over previous
"""GQA attention (B=1, S=2048, H=2048, 32 q-heads / 8 kv-heads, hd=64)
on 8 Trainium2 NeuronCores.

Sharding: tensor-parallel over heads for QKV+attention (core c owns
q-heads 4c..4c+3 and kv-head c), then sequence-parallel o_proj: per
1024-query round, two AllToAlls (one per head-pair) redistribute the
transposed, normalized attention output so core j owns query rows
{128j..128j+128, 1024+128j..+128}; each core holds the FULL wo (bf16)
and computes its 256 output rows locally. Host concatenates.

All large matmuls are bf16 (fp32r lowers to fp32_mode=HIGH at ~3
cycles/row on HW; bf16 runs at 1). Engine queues are FIFO in emission
order, so the program is emitted interleaved:
  warmup-MMs, A0, B0, A1, B1(kv), qc0, B1(q0), qc1(+round-0 A2As),
  B1(q1), qc2, round-0 o_proj, qc3(+round-1 A2As), round-1 o_proj
q/k/v and V_aug are split into per-1024-column-half tiles so qc0/qc1
depend only on half 0. Big const DMAs (wo, cos/sin) go on the GpSimd
queue; xT tiles alternate sync/scalar queues; a small AllToAll early
prewarms the collective path. Normalize+staging run per (q-chunk,
head-pair) so the tail only waits on the last chunk's chain.
"""
import numpy as np
import sys

sys.path.insert(0, "/opt/trn_rl_repo")

import concourse.bacc as bacc  # noqa: E402
import concourse.mybir as mybir  # noqa: E402
import concourse.tile as tile  # noqa: E402
from concourse import bass_utils  # noqa: E402

f32 = mybir.dt.float32
bf16 = mybir.dt.bfloat16
AF = mybir.ActivationFunctionType
BF16NP = mybir.dt.np(bf16)

N_CORES = 8
S = 2048
HID = 2048
HD = 64
ROPE_THETA = 10000.0
RMS_EPS = 1e-6
SCALING = HD ** -0.5              # 0.125
NK = HID // 128                   # 16 contraction tiles
NQC = S // 512                    # 4 q chunks
NKT = S // 128                    # 16 kpos tiles

_NC_CACHE = None
LAST_RESULTS = None


def _build():
    nc = bacc.Bacc("TRN2", target_bir_lowering=False, debug=False,
                   num_devices=N_CORES)

    def din(name, shape, dt):
        return nc.dram_tensor(name, shape, dt, kind="ExternalInput").ap()

    xT = din("xT", [HID, S], bf16)
    # host-pretiled: row p, col block t = original rows 128t+p
    wq0 = din("wq0", [128, HID], bf16)
    wq1 = din("wq1", [128, HID], bf16)
    wkv = din("wkv", [128, HID], bf16)     # [wv | wk] columns pretiled
    wof = din("wof", [128, NK * HID], bf16)  # FULL wo, pretiled
    cos2 = din("cos2", [128, S], bf16)
    ss2 = din("ss2", [128, S], bf16)
    ew_q = din("ew_q", [2, 128], bf16)
    ew_k = din("ew_k", [2, 128], bf16)
    e2 = din("e2", [2, 128], bf16)
    e2t = din("e2t", [128, 2], bf16)
    mask = din("mask", [128, 128], bf16)
    ident = din("ident", [64, 64], bf16)

    out_rs = nc.dram_tensor("out_rs", [256, S], f32,
                            kind="ExternalOutput").ap()

    with tile.TileContext(nc) as tc:
        with tc.tile_pool(name="consts", bufs=1) as cp, \
             tc.tile_pool(name="dram", bufs=1, space="DRAM") as dp:
            c_wq0 = cp.tile([128, HID], bf16, tag="w")
            c_wq1 = cp.tile([128, HID], bf16, tag="w2")
            c_wkv = cp.tile([128, HID], bf16, tag="w3")
            c_wo = cp.tile([128, NK * HID], bf16, tag="w4")
            c_cos = cp.tile([128, S], bf16, tag="c1")
            c_ss = cp.tile([128, S], bf16, tag="c2")
            c_ewq = cp.tile([2, 128], bf16, tag="c3")
            c_ewk = cp.tile([2, 128], bf16, tag="c4")
            c_e2 = cp.tile([2, 128], bf16, tag="c5")
            c_e2t = cp.tile([128, 2], bf16, tag="c5t")
            c_mask = cp.tile([128, 128], bf16, tag="c6")
            c_id = cp.tile([64, 64], bf16, tag="c7")
            c_eps = cp.tile([2, 1], f32, tag="c8")
            c_scr = cp.tile([128, 640], bf16, tag="c9")

            # PE warmup: memset scratch, then dummy matmuls so the HAM
            # clock gate is at 8/8 when the first real matmul lands.
            nc.vector.memset(c_scr[:], 0.0)
            nc.vector.memset(c_eps[:], RMS_EPS)

            # wq0 + even xT tiles on sync (gate the first matmuls);
            # wq1/wkv + odd xT tiles + small consts on scalar
            nc.sync.dma_start(c_wq0[:], wq0)
            nc.scalar.dma_start(c_wq1[:], wq1)
            nc.scalar.dma_start(c_wkv[:], wkv)
            nc.scalar.dma_start(c_e2t[:], e2t)
            nc.scalar.dma_start(c_ewq[:], ew_q)
            nc.scalar.dma_start(c_ewk[:], ew_k)
            nc.scalar.dma_start(c_id[:], ident)
            nc.scalar.dma_start(c_e2[:], e2)
            # rope tables + CDE consts + full wo on the gpsimd queue
            nc.gpsimd.dma_start(c_cos[:], cos2)
            nc.gpsimd.dma_start(c_ss[:], ss2)
            nc.gpsimd.dma_start(c_mask[:], mask)
            for h in range(8):
                cs_ = slice(4096 * h, 4096 * (h + 1))
                nc.gpsimd.dma_start(c_wo[:, cs_], wof[:, cs_])

            qkv = {
                "q0": cp.tile([128, S], bf16, tag="q0", name="q0"),
                "q1": cp.tile([128, S], bf16, tag="q1", name="q1"),
                "kv": cp.tile([128, S], bf16, tag="kv", name="kv"),
            }
            # per-half rope'd q/k and V_aug tiles
            qr0h = [cp.tile([128, 1024], bf16, tag=f"qr0{h}",
                            name=f"qr0{h}") for h in range(2)]
            qr1h = [cp.tile([128, 1024], bf16, tag=f"qr1{h}",
                            name=f"qr1{h}") for h in range(2)]
            krdh = [cp.tile([128, 1024], bf16, tag=f"krd{h}",
                            name=f"krd{h}") for h in range(2)]
            vah = [cp.tile([128, 8 * (HD + 1)], bf16, tag=f"va{h}",
                           name=f"va{h}") for h in range(2)]

            attn_bf = [cp.tile([128, S], bf16, tag=f"abf{i}",
                               name=f"abf{i}") for i in range(2)]
            l_sb = [cp.tile([2, S], bf16, tag=f"lsb{i}", name=f"lsb{i}")
                    for i in range(2)]

            a2a_in = [dp.tile([S, 128], bf16, name=f"a2ai{r}")
                      for r in range(2)]
            a2a_out = [dp.tile([S, 128], bf16, name=f"a2ao{r}")
                       for r in range(2)]

            # ================ phase A+B, halves ================
            with tc.tile_pool(name="xt", bufs=3) as xp, \
                 tc.tile_pool(name="sbB", bufs=2) as sbB:

                def phase_a(qh, psA, psM):
                    hs = slice(1024 * qh, 1024 * qh + 1024)
                    pq = [psA.tile([128, 1024], f32, tag="pa",
                                   name=f"pa{qh}_{j}") for j in range(3)]
                    for t in range(NK):
                        xt = xp.tile([128, 1024], bf16, tag="xt")
                        eng = nc.sync if t % 2 == 0 else nc.scalar
                        eng.dma_start(xt[:],
                                      xT[128 * t:128 * (t + 1), hs])
                        st = (t == 0)
                        sp = (t == NK - 1)
                        tc_ = slice(128 * t, 128 * (t + 1))
                        for j, w in ((0, c_wq0), (1, c_wq1), (2, c_wkv)):
                            nc.tensor.matmul(pq[j][:, 0:512], w[:, tc_],
                                             xt[:, 0:512],
                                             start=st, stop=sp)
                            nc.tensor.matmul(pq[j][:, 512:1024],
                                             w[:, tc_], xt[:, 512:1024],
                                             start=st, stop=sp)
                    for j, key in ((0, "q0"), (1, "q1"), (2, "kv")):
                        nc.vector.tensor_copy(qkv[key][:, hs], pq[j][:])

                def phase_b_spec(qh, si, key, ew, dst, is_kv, psM,
                                 ptag="m"):
                    hs = slice(1024 * qh, 1024 * qh + 1024)
                    src = qkv[key]
                    if is_kv:
                        nc.gpsimd.memset(vah[qh][:], 1.0)
                        for lt in range(8):
                            ptr = psM.tile([128, 64], bf16, tag=ptag,
                                           name=f"ptr{qh}_{lt}")
                            nc.tensor.transpose(
                                ptr[:],
                                src[0:64, 1024 * qh + 128 * lt:
                                    1024 * qh + 128 * (lt + 1)],
                                c_id[:])
                            nc.vector.tensor_copy(
                                vah[qh][:, (HD + 1) * lt:
                                        (HD + 1) * lt + HD],
                                ptr[:])
                    sq = sbB.tile([128, 1024], bf16, tag="sq",
                                  bufs=2, name=f"sq{qh}_{si}")
                    nc.vector.tensor_mul(sq[:], src[:, hs], src[:, hs])
                    rstds = {}
                    for u in range(2):
                        us = slice(512 * u, 512 * u + 512)
                        pss = psM.tile([2, 512], f32, tag=ptag,
                                       name=f"ss{qh}_{si}_{u}")
                        nc.tensor.matmul(pss[:], c_e2t[:], sq[:, us],
                                         start=True, stop=True)
                        lnv = sbB.tile([2, 512], bf16, tag="lnv",
                                       bufs=4, name=f"lnv{qh}{si}{u}")
                        nc.scalar.activation(lnv[:], pss[:], AF.Ln,
                                             scale=1.0 / HD,
                                             bias=c_eps[:])
                        rr = sbB.tile([2, 512], bf16, tag="rstdr",
                                      bufs=4, name=f"rr{qh}{si}{u}")
                        nc.scalar.activation(rr[:], lnv[:],
                                             AF.Exp, scale=-0.5)
                        rstds[u] = rr
                    rows = slice(64, 128) if is_kv else slice(0, 128)
                    nrm = sbB.tile([128, 1024], f32, tag="nrm",
                                   bufs=2, name=f"nrm{qh}_{si}")
                    for u in range(2):
                        cs = slice(1024 * qh + 512 * u,
                                   1024 * qh + 512 * u + 512)
                        us = slice(512 * u, 512 * u + 512)
                        pb = psM.tile([128, 512], f32, tag=ptag,
                                      name=f"pb{qh}_{si}_{u}")
                        nc.tensor.matmul(pb[:], ew[:], rstds[u][:],
                                         start=True, stop=True)
                        nc.vector.tensor_mul(nrm[rows, us],
                                             src[rows, cs], pb[rows, :])
                    # rope
                    sh = sbB.tile([128, 1024], f32, tag="sh",
                                  bufs=2, name=f"sh{qh}_{si}")
                    if is_kv:
                        nc.sync.dma_start(sh[64:96, :], nrm[96:128, :])
                        nc.sync.dma_start(sh[96:128, :], nrm[64:96, :])
                    else:
                        nc.sync.dma_start(sh[0:32, :], nrm[32:64, :])
                        nc.sync.dma_start(sh[32:64, :], nrm[0:32, :])
                        nc.sync.dma_start(sh[64:96, :], nrm[96:128, :])
                        nc.sync.dma_start(sh[96:128, :], nrm[64:96, :])
                    t2 = sbB.tile([128, 1024], f32, tag="t2",
                                  bufs=1, name=f"t2{qh}_{si}")
                    nc.vector.tensor_mul(t2[rows, :], sh[rows, :],
                                         c_ss[rows, hs])
                    t1 = sbB.tile([128, 1024], f32, tag="sh",
                                  bufs=2, name=f"t1{qh}_{si}")
                    nc.vector.tensor_mul(t1[rows, :], nrm[rows, :],
                                         c_cos[rows, hs])
                    nc.vector.tensor_add(dst[rows, :], t1[rows, :],
                                         t2[rows, :])
                    if is_kv:
                        nc.sync.dma_start(dst[0:64, :], dst[64:128, :])

                # ---- scope 1: warmup + A0 + B0 + A1 ----
                with tc.tile_pool(name="psA", bufs=3,
                                  space="PSUM") as psA, \
                     tc.tile_pool(name="psM", bufs=2,
                                  space="PSUM") as psM:
                    pwm = psM.tile([128, 512], f32, tag="m", name="pwm")
                    for i in range(40):
                        nc.tensor.matmul(pwm[:], c_scr[:, 0:128],
                                         c_scr[:, 128:640],
                                         start=True, stop=True)
                    phase_a(0, psA, psM)
                    for si, (key, ew, dst, ik) in enumerate((
                            ("kv", c_ewk, krdh[0], True),
                            ("q0", c_ewq, qr0h[0], False),
                            ("q1", c_ewq, qr1h[0], False))):
                        phase_b_spec(0, si, key, ew, dst, ik, psM)
                    phase_a(1, psA, psM)

                # ---- scope 2: B1 interleaved with CDE ----
                with tc.tile_pool(name="sbC", bufs=4) as sbC, \
                     tc.tile_pool(name="atk", bufs=2) as akp, \
                     tc.tile_pool(name="psS", bufs=2,
                                  space="PSUM") as psS, \
                     tc.tile_pool(name="psPV", bufs=2,
                                  space="PSUM") as psPV, \
                     tc.tile_pool(name="psO", bufs=2,
                                  space="PSUM") as psO:

                    def qchunk(qc):
                        qs = slice(512 * qc, 512 * qc + 512)
                        qhh = qc // 2
                        for hp, qrh in ((0, qr0h), (1, qr1h)):
                            qr = qrh[qhh]
                            ppv_a = psPV.tile([65, 512], f32, tag="pv")
                            ppv_b = psPV.tile([65, 512], f32, tag="pv")
                            ntile = 4 * qc + 4
                            for t in range(ntile):
                                r = t - 4 * qc
                                off = max(0, r) * 128
                                qlo = 512 * qc + off - 1024 * qhh
                                qlen = 512 * qc + 512 - 1024 * qhh - qlo
                                kh = t // 8
                                krd = krdh[kh]
                                v_aug = vah[kh]
                                tl = t - 8 * kh
                                kc = slice(128 * tl, 128 * (tl + 1))
                                vs = slice((HD + 1) * tl,
                                           (HD + 1) * tl + HD + 1)
                                st = (t == 0)
                                sp = (t == ntile - 1)
                                ps_s = psS.tile([128, 1024], f32,
                                                tag="s")
                                nc.tensor.matmul(
                                    ps_s[:, 0:qlen], krd[0:64, kc],
                                    qr[0:64, qlo:qlo + qlen],
                                    start=True, stop=True)
                                nc.tensor.matmul(
                                    ps_s[:, 512:512 + qlen],
                                    krd[64:128, kc],
                                    qr[64:128, qlo:qlo + qlen],
                                    start=True, stop=True)
                                pt = sbC.tile([128, 1024], bf16,
                                              tag="pt")
                                if r >= 0:
                                    nc.scalar.activation(
                                        pt[:, 0:512 + qlen],
                                        ps_s[:, 0:512 + qlen],
                                        AF.Exp, scale=SCALING)
                                    nc.vector.tensor_mul(
                                        pt[:, 0:128], pt[:, 0:128],
                                        c_mask[:])
                                    nc.vector.tensor_mul(
                                        pt[:, 512:640], pt[:, 512:640],
                                        c_mask[:])
                                else:
                                    nc.scalar.activation(
                                        pt[:, 0:1024], ps_s[:, 0:1024],
                                        AF.Exp, scale=SCALING)
                                nc.tensor.matmul(
                                    ppv_a[:, off:512], v_aug[:, vs],
                                    pt[:, 0:qlen], start=st, stop=sp)
                                nc.tensor.matmul(
                                    ppv_b[:, off:512], v_aug[:, vs],
                                    pt[:, 512:512 + qlen],
                                    start=st, stop=sp)
                            # per-hp: stage, 1/l, normalize, a2a-stage
                            for half, ppv in ((0, ppv_a), (1, ppv_b)):
                                stg = sbC.tile([65, 512], bf16,
                                               tag="stg", bufs=3)
                                nc.vector.tensor_copy(stg[:], ppv[:])
                                nc.sync.dma_start(
                                    attn_bf[hp][64 * half:
                                                64 * half + 64, qs],
                                    stg[0:64, :])
                                nc.sync.dma_start(
                                    l_sb[hp][half:half + 1, qs],
                                    stg[64:65, :])
                            rl = sbC.tile([2, 512], f32, tag="lnl",
                                          bufs=2, name=f"rl{qc}_{hp}")
                            nc.vector.reciprocal(rl[:],
                                                 l_sb[hp][:, qs])
                            rlb = sbC.tile([2, 512], bf16, tag="rlb",
                                           bufs=2, name=f"rb{qc}_{hp}")
                            nc.vector.tensor_copy(rlb[:], rl[:])
                            pb = psO.tile([128, 512], f32, tag="o")
                            nc.tensor.matmul(pb[:], c_e2[:], rlb[:],
                                             start=True, stop=True)
                            nc.vector.tensor_mul(
                                attn_bf[hp][:, qs],
                                attn_bf[hp][:, qs], pb[:])
                            rnd = qc // 2
                            seng = nc.gpsimd if hp == 0 else nc.scalar
                            for j in range(4 * (qc % 2),
                                           4 * (qc % 2) + 4):
                                qq = 1024 * rnd + 128 * j
                                rr_ = 256 * j + 128 * hp
                                seng.dma_start(
                                    a2a_in[rnd][rr_:rr_ + 128, :],
                                    attn_bf[hp][:, qq:qq + 128])
                        if qc % 2 == 1:
                            rnd = qc // 2
                            nc.gpsimd.collective_compute(
                                "AllToAll",
                                mybir.AluOpType.bypass,
                                replica_groups=[list(range(N_CORES))],
                                ins=[a2a_in[rnd][:, :].opt()],
                                outs=[a2a_out[rnd][:, :].opt()],
                            )

                    def oproj(rnd):
                        attk = akp.tile([128, S], bf16, tag="atk")
                        for kk in range(NK):
                            eng = nc.gpsimd if kk % 2 == 0 else \
                                nc.scalar
                            eng.dma_start(
                                attk[:, 128 * kk:128 * (kk + 1)],
                                a2a_out[rnd][128 * kk:
                                             128 * (kk + 1), :])
                        pos = [psO.tile([128, 512], f32, tag="o",
                                        name=f"po{rnd}_0"),
                               psO.tile([128, 512], f32, tag="o",
                                        name=f"po{rnd}_1"),
                               psS.tile([128, 512], f32, tag="s",
                                        name=f"po{rnd}_2"),
                               psS.tile([128, 512], f32, tag="s",
                                        name=f"po{rnd}_3")]
                        for kk in range(NK):
                            for n in range(4):
                                nc.tensor.matmul(
                                    pos[n][:],
                                    attk[:, 128 * kk:128 * (kk + 1)],
                                    c_wo[:, HID * kk + 512 * n:
                                         HID * kk + 512 * n + 512],
                                    start=(kk == 0),
                                    stop=(kk == NK - 1))
                        for n in range(4):
                            ost = sbC.tile([128, 512], f32, tag="ost",
                                           bufs=2)
                            nc.vector.tensor_copy(ost[:], pos[n][:])
                            nc.sync.dma_start(
                                out_rs[128 * rnd:128 * rnd + 128,
                                       512 * n:512 * n + 512],
                                ost[:])

                    phase_b_spec(1, 0, "kv", c_ewk, krdh[1], True,
                                 psO, ptag="o")
                    qchunk(0)
                    phase_b_spec(1, 1, "q0", c_ewq, qr0h[1], False,
                                 psO, ptag="o")
                    qchunk(1)
                    phase_b_spec(1, 2, "q1", c_ewq, qr1h[1], False,
                                 psO, ptag="o")
                    qchunk(2)
                    oproj(0)
                    qchunk(3)
                    oproj(1)

    nc.compile()
    return nc


def _host_prep(hidden_states, position_ids, wq, wk, wv, wo, q_ln_w, k_ln_w):
    x = np.asarray(hidden_states, dtype=np.float32)[0]        # [S, HID]
    xT = np.ascontiguousarray(x.T).astype(BF16NP)             # [HID, S]
    pos = np.asarray(position_ids)[0].astype(np.float32)      # [S]
    inv = 1.0 / (ROPE_THETA ** (np.arange(0, HD, 2, dtype=np.float32) / HD))
    ang = pos[:, None] * inv[None, :]                         # [S, 32]
    emb = np.concatenate([ang, ang], axis=1)                  # [S, 64]
    cosT = np.cos(emb).T.astype(np.float32)                   # [64, S]
    sinT = np.sin(emb).T.astype(np.float32)
    ss = sinT.copy()
    ss[0:32] = -sinT[0:32]
    cos2 = np.tile(cosT, (2, 1))
    ss2 = np.tile(ss, (2, 1))

    e2 = np.zeros((2, 128), dtype=np.float32)
    e2[0, 0:64] = 1.0
    e2[1, 64:128] = 1.0
    ew_q = np.zeros((2, 128), dtype=np.float32)
    ew_q[0, 0:64] = q_ln_w
    ew_q[1, 64:128] = q_ln_w
    ew_k = np.zeros((2, 128), dtype=np.float32)
    ew_k[1, 64:128] = k_ln_w
    msk = (np.arange(128)[:, None] <= np.arange(128)[None, :]) \
        .astype(np.float32)
    ident = np.eye(64, dtype=np.float32)

    wq_ = np.asarray(wq, dtype=np.float32)
    wk_ = np.asarray(wk, dtype=np.float32)
    wv_ = np.asarray(wv, dtype=np.float32)
    wo_ = np.asarray(wo, dtype=np.float32)

    def pretile(w):  # [HID, N] -> [128, NK*N] ktile-blocked
        n = w.shape[1]
        return np.ascontiguousarray(
            w.reshape(NK, 128, n).transpose(1, 0, 2).reshape(128, NK * n))

    wof = pretile(wo_).astype(BF16NP)

    in_maps = []
    for c in range(N_CORES):
        qcols = slice(256 * c, 256 * (c + 1))
        kvcols = slice(64 * c, 64 * (c + 1))
        wq_c = np.ascontiguousarray(wq_[:, qcols])
        wkv_c = np.concatenate([wv_[:, kvcols], wk_[:, kvcols]], axis=1)
        in_maps.append({
            "xT": xT,
            "wq0": pretile(wq_c[:, 0:128]).astype(BF16NP),
            "wq1": pretile(wq_c[:, 128:256]).astype(BF16NP),
            "wkv": pretile(wkv_c).astype(BF16NP),
            "wof": wof,
            "cos2": cos2.astype(BF16NP),
            "ss2": ss2.astype(BF16NP),
            "ew_q": ew_q.astype(BF16NP),
            "ew_k": ew_k.astype(BF16NP),
            "e2": e2.astype(BF16NP),
            "e2t": np.ascontiguousarray(e2.T).astype(BF16NP),
            "mask": msk.astype(BF16NP),
            "ident": ident.astype(BF16NP),
        })
    return in_maps


def kernel(hidden_states, position_ids, wq, wk, wv, wo, q_ln_w, k_ln_w):
    global _NC_CACHE, LAST_RESULTS
    if _NC_CACHE is None:
        _NC_CACHE = _build()
    nc = _NC_CACHE
    in_maps = _host_prep(hidden_states, position_ids, wq, wk, wv, wo,
                         q_ln_w, k_ln_w)
    res = bass_utils.run_bass_kernel_spmd(
        nc, in_maps, core_ids=list(range(N_CORES)))
    LAST_RESULTS = res
    out = np.empty((S, HID), dtype=np.float32)
    for c in range(N_CORES):
        o_c = res.results[c]["out_rs"]        # [256, 2048]
        for rnd in range(2):
            out[1024 * rnd + 128 * c:1024 * rnd + 128 * c + 128, :] = \
                o_c[128 * rnd:128 * rnd + 128, :]
    return out.reshape(1, S, HID)


# revision 18
# speedup vs baseline: 1.1955x; 1.0458x over previous
"""GQA attention (B=1, S=2048, H=2048, 32 q-heads / 8 kv-heads, hd=64)
on 8 Trainium2 NeuronCores.

Sharding: tensor-parallel over heads for QKV+attention (core c owns
q-heads 4c..4c+3 and kv-head c), then sequence-parallel o_proj: per
1024-query round, two AllToAlls (one per head-pair) redistribute the
transposed, normalized attention output so core j owns query rows
{128j..128j+128, 1024+128j..+128}; each core holds the FULL wo (bf16)
and computes its 256 output rows locally. Host concatenates.

All large matmuls are bf16 (fp32r lowers to fp32_mode=HIGH at ~3
cycles/row on HW; bf16 runs at 1). Engine queues are FIFO in emission
order, so the program is emitted interleaved:
  warmup-MMs, A0, B0, A1, B1(kv), qc0, B1(q0), qc1(+round-0 A2As),
  B1(q1), qc2, round-0 o_proj, qc3(+round-1 A2As), round-1 o_proj
q/k/v and V_aug are split into per-1024-column-half tiles so qc0/qc1
depend only on half 0. Big const DMAs (wo, cos/sin) go on the GpSimd
queue; xT tiles alternate sync/scalar queues; a small AllToAll early
prewarms the collective path. Normalize+staging run per (q-chunk,
head-pair) so the tail only waits on the last chunk's chain.
"""
import numpy as np
import sys

sys.path.insert(0, "/opt/trn_rl_repo")

import concourse.bacc as bacc  # noqa: E402
import concourse.mybir as mybir  # noqa: E402
import concourse.tile as tile  # noqa: E402
from concourse import bass_utils  # noqa: E402

f32 = mybir.dt.float32
bf16 = mybir.dt.bfloat16
AF = mybir.ActivationFunctionType
BF16NP = mybir.dt.np(bf16)

N_CORES = 8
S = 2048
HID = 2048
HD = 64
ROPE_THETA = 10000.0
RMS_EPS = 1e-6
SCALING = HD ** -0.5              # 0.125
NK = HID // 128                   # 16 contraction tiles
NQC = S // 512                    # 4 q chunks
NKT = S // 128                    # 16 kpos tiles

_NC_CACHE = None
LAST_RESULTS = None


def _build():
    nc = bacc.Bacc("TRN2", target_bir_lowering=False, debug=False,
                   num_devices=N_CORES)

    def din(name, shape, dt):
        return nc.dram_tensor(name, shape, dt, kind="ExternalInput").ap()

    xT = din("xT", [HID, S], bf16)
    # host-pretiled: row p, col block t = original rows 128t+p
    wq0 = din("wq0", [128, HID], bf16)
    wq1 = din("wq1", [128, HID], bf16)
    wkv = din("wkv", [128, HID], bf16)     # [wv | wk] columns pretiled
    wof = din("wof", [128, NK * HID], bf16)  # FULL wo, pretiled
    cos2 = din("cos2", [128, S], bf16)
    ss2 = din("ss2", [128, S], bf16)
    ew_q = din("ew_q", [2, 128], bf16)
    ew_k = din("ew_k", [2, 128], bf16)
    e2 = din("e2", [2, 128], bf16)
    e2t = din("e2t", [128, 2], bf16)
    mask = din("mask", [128, 128], bf16)
    ident = din("ident", [64, 64], bf16)

    out_rs = nc.dram_tensor("out_rs", [256, S], f32,
                            kind="ExternalOutput").ap()

    with tile.TileContext(nc) as tc:
        with tc.tile_pool(name="consts", bufs=1) as cp, \
             tc.tile_pool(name="dram", bufs=1, space="DRAM") as dp:
            c_wq0 = cp.tile([128, HID], bf16, tag="w")
            c_wq1 = cp.tile([128, HID], bf16, tag="w2")
            c_wkv = cp.tile([128, HID], bf16, tag="w3")
            c_wo = cp.tile([128, NK * HID], bf16, tag="w4")
            c_cos = cp.tile([128, S], bf16, tag="c1")
            c_ss = cp.tile([128, S], bf16, tag="c2")
            c_ewq = cp.tile([2, 128], bf16, tag="c3")
            c_ewk = cp.tile([2, 128], bf16, tag="c4")
            c_e2 = cp.tile([2, 128], bf16, tag="c5")
            c_e2t = cp.tile([128, 2], bf16, tag="c5t")
            c_mask = cp.tile([128, 128], bf16, tag="c6")
            c_id = cp.tile([64, 64], bf16, tag="c7")
            c_eps = cp.tile([2, 1], f32, tag="c8")
            c_scr = cp.tile([128, 640], bf16, tag="c9")

            # PE warmup: memset scratch, then dummy matmuls so the HAM
            # clock gate is at 8/8 when the first real matmul lands.
            nc.vector.memset(c_scr[:], 0.0)
            nc.vector.memset(c_eps[:], RMS_EPS)

            # wq0 + even xT tiles on sync (gate the first matmuls);
            # wq1/wkv + odd xT tiles + small consts on scalar
            for h in range(4):
                cs_ = slice(512 * h, 512 * (h + 1))
                nc.sync.dma_start(c_wq0[:, cs_], wq0[:, cs_])
            nc.scalar.dma_start(c_wq1[:], wq1)
            nc.scalar.dma_start(c_wkv[:], wkv)
            nc.scalar.dma_start(c_e2t[:], e2t)
            nc.scalar.dma_start(c_ewq[:], ew_q)
            nc.scalar.dma_start(c_ewk[:], ew_k)
            nc.scalar.dma_start(c_id[:], ident)
            nc.scalar.dma_start(c_e2[:], e2)
            # rope tables + CDE consts + full wo on the gpsimd queue
            nc.gpsimd.dma_start(c_cos[:], cos2)
            nc.gpsimd.dma_start(c_ss[:], ss2)
            nc.gpsimd.dma_start(c_mask[:], mask)

            qkv = {
                "q0": cp.tile([128, S], bf16, tag="q0", name="q0"),
                "q1": cp.tile([128, S], bf16, tag="q1", name="q1"),
                "kv": cp.tile([128, S], bf16, tag="kv", name="kv"),
            }
            # per-half rope'd q/k and V_aug tiles
            qr0h = [cp.tile([128, 1024], bf16, tag=f"qr0{h}",
                            name=f"qr0{h}") for h in range(2)]
            qr1h = [cp.tile([128, 1024], bf16, tag=f"qr1{h}",
                            name=f"qr1{h}") for h in range(2)]
            krdh = [cp.tile([128, 1024], bf16, tag=f"krd{h}",
                            name=f"krd{h}") for h in range(2)]
            vah = [cp.tile([128, 8 * (HD + 1)], bf16, tag=f"va{h}",
                           name=f"va{h}") for h in range(2)]

            attn_bf = [cp.tile([128, S], bf16, tag=f"abf{i}",
                               name=f"abf{i}") for i in range(2)]
            l_sb = [cp.tile([2, S], bf16, tag=f"lsb{i}", name=f"lsb{i}")
                    for i in range(2)]

            a2a_in = [dp.tile([S, 128], bf16, name=f"a2ai{r}")
                      for r in range(2)]
            a2a_out = [dp.tile([S, 128], bf16, name=f"a2ao{r}")
                       for r in range(2)]

            # ================ phase A+B, halves ================
            with tc.tile_pool(name="xt", bufs=3) as xp, \
                 tc.tile_pool(name="sbB", bufs=2) as sbB:

                def phase_a(qh, psA, psM):
                    hs = slice(1024 * qh, 1024 * qh + 1024)
                    pq = [psA.tile([128, 1024], f32, tag="pa",
                                   name=f"pa{qh}_{j}") for j in range(3)]
                    for t in range(NK):
                        xt = xp.tile([128, 1024], bf16, tag="xt")
                        eng = (nc.sync, nc.scalar, nc.gpsimd)[t % 3]
                        eng.dma_start(xt[:],
                                      xT[128 * t:128 * (t + 1), hs])
                        st = (t == 0)
                        sp = (t == NK - 1)
                        tc_ = slice(128 * t, 128 * (t + 1))
                        for j, w in ((0, c_wq0), (1, c_wq1), (2, c_wkv)):
                            nc.tensor.matmul(pq[j][:, 0:512], w[:, tc_],
                                             xt[:, 0:512],
                                             start=st, stop=sp)
                            nc.tensor.matmul(pq[j][:, 512:1024],
                                             w[:, tc_], xt[:, 512:1024],
                                             start=st, stop=sp)
                    for j, key in ((0, "q0"), (1, "q1"), (2, "kv")):
                        nc.vector.tensor_copy(qkv[key][:, hs], pq[j][:])

                def phase_b_spec(qh, si, key, ew, dst, is_kv, psM,
                                 ptag="m"):
                    hs = slice(1024 * qh, 1024 * qh + 1024)
                    src = qkv[key]
                    if is_kv:
                        nc.gpsimd.memset(vah[qh][:], 1.0)
                        for lt in range(8):
                            ptr = psM.tile([128, 64], bf16, tag=ptag,
                                           name=f"ptr{qh}_{lt}")
                            nc.tensor.transpose(
                                ptr[:],
                                src[0:64, 1024 * qh + 128 * lt:
                                    1024 * qh + 128 * (lt + 1)],
                                c_id[:])
                            nc.vector.tensor_copy(
                                vah[qh][:, (HD + 1) * lt:
                                        (HD + 1) * lt + HD],
                                ptr[:])
                    sq = sbB.tile([128, 1024], bf16, tag="sq",
                                  bufs=2, name=f"sq{qh}_{si}")
                    nc.vector.tensor_mul(sq[:], src[:, hs], src[:, hs])
                    rstds = {}
                    for u in range(2):
                        us = slice(512 * u, 512 * u + 512)
                        pss = psM.tile([2, 512], f32, tag=ptag,
                                       name=f"ss{qh}_{si}_{u}")
                        nc.tensor.matmul(pss[:], c_e2t[:], sq[:, us],
                                         start=True, stop=True)
                        lnv = sbB.tile([2, 512], bf16, tag="lnv",
                                       bufs=4, name=f"lnv{qh}{si}{u}")
                        nc.scalar.activation(lnv[:], pss[:], AF.Ln,
                                             scale=1.0 / HD,
                                             bias=c_eps[:])
                        rr = sbB.tile([2, 512], bf16, tag="rstdr",
                                      bufs=4, name=f"rr{qh}{si}{u}")
                        nc.scalar.activation(rr[:], lnv[:],
                                             AF.Exp, scale=-0.5)
                        rstds[u] = rr
                    rows = slice(64, 128) if is_kv else slice(0, 128)
                    nrm = sbB.tile([128, 1024], f32, tag="nrm",
                                   bufs=2, name=f"nrm{qh}_{si}")
                    for u in range(2):
                        cs = slice(1024 * qh + 512 * u,
                                   1024 * qh + 512 * u + 512)
                        us = slice(512 * u, 512 * u + 512)
                        pb = psM.tile([128, 512], f32, tag=ptag,
                                      name=f"pb{qh}_{si}_{u}")
                        nc.tensor.matmul(pb[:], ew[:], rstds[u][:],
                                         start=True, stop=True)
                        nc.vector.tensor_mul(nrm[rows, us],
                                             src[rows, cs], pb[rows, :])
                    # rope
                    sh = sbB.tile([128, 1024], f32, tag="sh",
                                  bufs=2, name=f"sh{qh}_{si}")
                    if is_kv:
                        nc.sync.dma_start(sh[64:96, :], nrm[96:128, :])
                        nc.sync.dma_start(sh[96:128, :], nrm[64:96, :])
                    else:
                        nc.sync.dma_start(sh[0:32, :], nrm[32:64, :])
                        nc.sync.dma_start(sh[32:64, :], nrm[0:32, :])
                        nc.sync.dma_start(sh[64:96, :], nrm[96:128, :])
                        nc.sync.dma_start(sh[96:128, :], nrm[64:96, :])
                    t2 = sbB.tile([128, 1024], f32, tag="t2",
                                  bufs=1, name=f"t2{qh}_{si}")
                    nc.vector.tensor_mul(t2[rows, :], sh[rows, :],
                                         c_ss[rows, hs])
                    t1 = sbB.tile([128, 1024], f32, tag="sh",
                                  bufs=2, name=f"t1{qh}_{si}")
                    nc.vector.tensor_mul(t1[rows, :], nrm[rows, :],
                                         c_cos[rows, hs])
                    nc.vector.tensor_add(dst[rows, :], t1[rows, :],
                                         t2[rows, :])
                    if is_kv:
                        nc.sync.dma_start(dst[0:64, :], dst[64:128, :])

                # ---- scope 1: warmup + A0 + B0 + A1 ----
                with tc.tile_pool(name="psA", bufs=3,
                                  space="PSUM") as psA, \
                     tc.tile_pool(name="psM", bufs=2,
                                  space="PSUM") as psM:
                    pwm = psM.tile([128, 512], f32, tag="m", name="pwm")
                    for i in range(40):
                        nc.tensor.matmul(pwm[:], c_scr[:, 0:128],
                                         c_scr[:, 128:640],
                                         start=True, stop=True)
                    phase_a(0, psA, psM)
                    for si, (key, ew, dst, ik) in enumerate((
                            ("kv", c_ewk, krdh[0], True),
                            ("q0", c_ewq, qr0h[0], False),
                            ("q1", c_ewq, qr1h[0], False))):
                        phase_b_spec(0, si, key, ew, dst, ik, psM)
                    phase_a(1, psA, psM)
                    for h in range(8):
                        cs_ = slice(4096 * h, 4096 * (h + 1))
                        nc.gpsimd.dma_start(c_wo[:, cs_], wof[:, cs_])

                # ---- scope 2: B1 interleaved with CDE ----
                with tc.tile_pool(name="sbC", bufs=4) as sbC, \
                     tc.tile_pool(name="atk", bufs=2) as akp, \
                     tc.tile_pool(name="psS", bufs=2,
                                  space="PSUM") as psS, \
                     tc.tile_pool(name="psPV", bufs=2,
                                  space="PSUM") as psPV, \
                     tc.tile_pool(name="psO", bufs=2,
                                  space="PSUM") as psO:

                    def qchunk(qc):
                        qs = slice(512 * qc, 512 * qc + 512)
                        qhh = qc // 2
                        for hp, qrh in ((0, qr0h), (1, qr1h)):
                            qr = qrh[qhh]
                            ppv_a = psPV.tile([65, 512], f32, tag="pv")
                            ppv_b = psPV.tile([65, 512], f32, tag="pv")
                            ntile = 4 * qc + 4
                            for t in range(ntile):
                                r = t - 4 * qc
                                off = max(0, r) * 128
                                qlo = 512 * qc + off - 1024 * qhh
                                qlen = 512 * qc + 512 - 1024 * qhh - qlo
                                kh = t // 8
                                krd = krdh[kh]
                                v_aug = vah[kh]
                                tl = t - 8 * kh
                                kc = slice(128 * tl, 128 * (tl + 1))
                                vs = slice((HD + 1) * tl,
                                           (HD + 1) * tl + HD + 1)
                                st = (t == 0)
                                sp = (t == ntile - 1)
                                ps_s = psS.tile([128, 1024], f32,
                                                tag="s")
                                nc.tensor.matmul(
                                    ps_s[:, 0:qlen], krd[0:64, kc],
                                    qr[0:64, qlo:qlo + qlen],
                                    start=True, stop=True)
                                nc.tensor.matmul(
                                    ps_s[:, 512:512 + qlen],
                                    krd[64:128, kc],
                                    qr[64:128, qlo:qlo + qlen],
                                    start=True, stop=True)
                                pt = sbC.tile([128, 1024], bf16,
                                              tag="pt")
                                if r >= 0:
                                    nc.scalar.activation(
                                        pt[:, 0:512 + qlen],
                                        ps_s[:, 0:512 + qlen],
                                        AF.Exp, scale=SCALING)
                                    nc.vector.tensor_mul(
                                        pt[:, 0:128], pt[:, 0:128],
                                        c_mask[:])
                                    nc.vector.tensor_mul(
                                        pt[:, 512:640], pt[:, 512:640],
                                        c_mask[:])
                                else:
                                    nc.scalar.activation(
                                        pt[:, 0:1024], ps_s[:, 0:1024],
                                        AF.Exp, scale=SCALING)
                                nc.tensor.matmul(
                                    ppv_a[:, off:512], v_aug[:, vs],
                                    pt[:, 0:qlen], start=st, stop=sp)
                                nc.tensor.matmul(
                                    ppv_b[:, off:512], v_aug[:, vs],
                                    pt[:, 512:512 + qlen],
                                    start=st, stop=sp)
                            # per-hp: stage, 1/l, normalize, a2a-stage
                            for half, ppv in ((0, ppv_a), (1, ppv_b)):
                                stg = sbC.tile([65, 512], bf16,
                                               tag="stg", bufs=3)
                                nc.vector.tensor_copy(stg[:], ppv[:])
                                nc.sync.dma_start(
                                    attn_bf[hp][64 * half:
                                                64 * half + 64, qs],
                                    stg[0:64, :])
                                nc.sync.dma_start(
                                    l_sb[hp][half:half + 1, qs],
                                    stg[64:65, :])
                            rl = sbC.tile([2, 512], f32, tag="lnl",
                                          bufs=2, name=f"rl{qc}_{hp}")
                            nc.vector.reciprocal(rl[:],
                                                 l_sb[hp][:, qs])
                            rlb = sbC.tile([2, 512], bf16, tag="rlb",
                                           bufs=2, name=f"rb{qc}_{hp}")
                            nc.vector.tensor_copy(rlb[:], rl[:])
                            pb = psO.tile([128, 512], f32, tag="o")
                            nc.tensor.matmul(pb[:], c_e2[:], rlb[:],
                                             start=True, stop=True)
                            nc.vector.tensor_mul(
                                attn_bf[hp][:, qs],
                                attn_bf[hp][:, qs], pb[:])
                            rnd = qc // 2
                            seng = nc.gpsimd if hp == 0 else nc.sync
                            for j in range(4 * (qc % 2),
                                           4 * (qc % 2) + 4):
                                qq = 1024 * rnd + 128 * j
                                rr_ = 256 * j + 128 * hp
                                seng.dma_start(
                                    a2a_in[rnd][rr_:rr_ + 128, :],
                                    attn_bf[hp][:, qq:qq + 128])
                        if qc % 2 == 1:
                            rnd = qc // 2
                            nc.gpsimd.collective_compute(
                                "AllToAll",
                                mybir.AluOpType.bypass,
                                replica_groups=[list(range(N_CORES))],
                                ins=[a2a_in[rnd][:, :].opt()],
                                outs=[a2a_out[rnd][:, :].opt()],
                            )

                    def oproj(rnd):
                        attk = akp.tile([128, S], bf16, tag="atk")
                        for kk in range(NK):
                            eng = nc.gpsimd if kk % 2 == 0 else \
                                nc.sync
                            eng.dma_start(
                                attk[:, 128 * kk:128 * (kk + 1)],
                                a2a_out[rnd][128 * kk:
                                             128 * (kk + 1), :])
                        pos = [psO.tile([128, 512], f32, tag="o",
                                        name=f"po{rnd}_0"),
                               psO.tile([128, 512], f32, tag="o",
                                        name=f"po{rnd}_1"),
                               psS.tile([128, 512], f32, tag="s",
                                        name=f"po{rnd}_2"),
                               psS.tile([128, 512], f32, tag="s",
                                        name=f"po{rnd}_3")]
                        for kk in range(NK):
                            for n in range(4):
                                nc.tensor.matmul(
                                    pos[n][:],
                                    attk[:, 128 * kk:128 * (kk + 1)],
                                    c_wo[:, HID * kk + 512 * n:
                                         HID * kk + 512 * n + 512],
                                    start=(kk == 0),
                                    stop=(kk == NK - 1))
                        for n in range(4):
                            ost = sbC.tile([128, 512], f32, tag="ost",
                                           bufs=2)
                            nc.vector.tensor_copy(ost[:], pos[n][:])
                            nc.sync.dma_start(
                                out_rs[128 * rnd:128 * rnd + 128,
                                       512 * n:512 * n + 512],
                                ost[:])

                    phase_b_spec(1, 0, "kv", c_ewk, krdh[1], True,
                                 psO, ptag="o")
                    qchunk(0)
                    phase_b_spec(1, 1, "q0", c_ewq, qr0h[1], False,
                                 psO, ptag="o")
                    qchunk(1)
                    phase_b_spec(1, 2, "q1", c_ewq, qr1h[1], False,
                                 psO, ptag="o")
                    qchunk(2)
                    qchunk(3)
                    oproj(0)
                    oproj(1)

    nc.compile()
    return nc


def _host_prep(hidden_states, position_ids, wq, wk, wv, wo, q_ln_w, k_ln_w):
    x = np.asarray(hidden_states, dtype=np.float32)[0]        # [S, HID]
    xT = np.ascontiguousarray(x.T).astype(BF16NP)             # [HID, S]
    pos = np.asarray(position_ids)[0].astype(np.float32)      # [S]
    inv = 1.0 / (ROPE_THETA ** (np.arange(0, HD, 2, dtype=np.float32) / HD))
    ang = pos[:, None] * inv[None, :]                         # [S, 32]
    emb = np.concatenate([ang, ang], axis=1)                  # [S, 64]
    cosT = np.cos(emb).T.astype(np.float32)                   # [64, S]
    sinT = np.sin(emb).T.astype(np.float32)
    ss = sinT.copy()
    ss[0:32] = -sinT[0:32]
    cos2 = np.tile(cosT, (2, 1))
    ss2 = np.tile(ss, (2, 1))

    e2 = np.zeros((2, 128), dtype=np.float32)
    e2[0, 0:64] = 1.0
    e2[1, 64:128] = 1.0
    ew_q = np.zeros((2, 128), dtype=np.float32)
    ew_q[0, 0:64] = q_ln_w
    ew_q[1, 64:128] = q_ln_w
    ew_k = np.zeros((2, 128), dtype=np.float32)
    ew_k[1, 64:128] = k_ln_w
    msk = (np.arange(128)[:, None] <= np.arange(128)[None, :]) \
        .astype(np.float32)
    ident = np.eye(64, dtype=np.float32)

    wq_ = np.asarray(wq, dtype=np.float32)
    wk_ = np.asarray(wk, dtype=np.float32)
    wv_ = np.asarray(wv, dtype=np.float32)
    wo_ = np.asarray(wo, dtype=np.float32)

    def pretile(w):  # [HID, N] -> [128, NK*N] ktile-blocked
        n = w.shape[1]
        return np.ascontiguousarray(
            w.reshape(NK, 128, n).transpose(1, 0, 2).reshape(128, NK * n))

    wof = pretile(wo_).astype(BF16NP)

    in_maps = []
    for c in range(N_CORES):
        qcols = slice(256 * c, 256 * (c + 1))
        kvcols = slice(64 * c, 64 * (c + 1))
        wq_c = np.ascontiguousarray(wq_[:, qcols])
        wkv_c = np.concatenate([wv_[:, kvcols], wk_[:, kvcols]], axis=1)
        in_maps.append({
            "xT": xT,
            "wq0": pretile(wq_c[:, 0:128]).astype(BF16NP),
            "wq1": pretile(wq_c[:, 128:256]).astype(BF16NP),
            "wkv": pretile(wkv_c).astype(BF16NP),
            "wof": wof,
            "cos2": cos2.astype(BF16NP),
            "ss2": ss2.astype(BF16NP),
            "ew_q": ew_q.astype(BF16NP),
            "ew_k": ew_k.astype(BF16NP),
            "e2": e2.astype(BF16NP),
            "e2t": np.ascontiguousarray(e2.T).astype(BF16NP),
            "mask": msk.astype(BF16NP),
            "ident": ident.astype(BF16NP),
        })
    return in_maps


def kernel(hidden_states, position_ids, wq, wk, wv, wo, q_ln_w, k_ln_w):
    global _NC_CACHE, LAST_RESULTS
    if _NC_CACHE is None:
        _NC_CACHE = _build()
    nc = _NC_CACHE
    in_maps = _host_prep(hidden_states, position_ids, wq, wk, wv, wo,
                         q_ln_w, k_ln_w)
    res = bass_utils.run_bass_kernel_spmd(
        nc, in_maps, core_ids=list(range(N_CORES)))
    LAST_RESULTS = res
    out = np.empty((S, HID), dtype=np.float32)
    for c in range(N_CORES):
        o_c = res.results[c]["out_rs"]        # [256, 2048]
        for rnd in range(2):
            out[1024 * rnd + 128 * c:1024 * rnd + 128 * c + 128, :] = \
                o_c[128 * rnd:128 * rnd + 128, :]
    return out.reshape(1, S, HID)


# revision 19
# speedup vs baseline: 1.2551x; 1.0498x over previous
"""GQA attention (B=1, S=2048, H=2048, 32 q-heads / 8 kv-heads, hd=64)
on 8 Trainium2 NeuronCores.

Sharding: tensor-parallel over heads for QKV+attention (core c owns
q-heads 4c..4c+3 and kv-head c), then sequence-parallel o_proj: per
1024-query round, two AllToAlls (one per head-pair) redistribute the
transposed, normalized attention output so core j owns query rows
{128j..128j+128, 1024+128j..+128}; each core holds the FULL wo (bf16)
and computes its 256 output rows locally. Host concatenates.

All large matmuls are bf16 (fp32r lowers to fp32_mode=HIGH at ~3
cycles/row on HW; bf16 runs at 1). Engine queues are FIFO in emission
order, so the program is emitted interleaved:
  warmup-MMs, A0, B0, A1, B1(kv), qc0, B1(q0), qc1(+round-0 A2As),
  B1(q1), qc2, round-0 o_proj, qc3(+round-1 A2As), round-1 o_proj
q/k/v and V_aug are split into per-1024-column-half tiles so qc0/qc1
depend only on half 0. Big const DMAs (wo, cos/sin) go on the GpSimd
queue; xT tiles alternate sync/scalar queues; a small AllToAll early
prewarms the collective path. Normalize+staging run per (q-chunk,
head-pair) so the tail only waits on the last chunk's chain.
"""
import numpy as np
import sys

sys.path.insert(0, "/opt/trn_rl_repo")

import concourse.bacc as bacc  # noqa: E402
import concourse.mybir as mybir  # noqa: E402
import concourse.tile as tile  # noqa: E402
from concourse import bass_utils  # noqa: E402

f32 = mybir.dt.float32
bf16 = mybir.dt.bfloat16
AF = mybir.ActivationFunctionType
BF16NP = mybir.dt.np(bf16)

N_CORES = 8
S = 2048
HID = 2048
HD = 64
ROPE_THETA = 10000.0
RMS_EPS = 1e-6
SCALING = HD ** -0.5              # 0.125
NK = HID // 128                   # 16 contraction tiles
NQC = S // 512                    # 4 q chunks
NKT = S // 128                    # 16 kpos tiles

_NC_CACHE = None
LAST_RESULTS = None


def _build():
    nc = bacc.Bacc("TRN2", target_bir_lowering=False, debug=False,
                   num_devices=N_CORES)

    def din(name, shape, dt):
        return nc.dram_tensor(name, shape, dt, kind="ExternalInput").ap()

    xP = din("xP", [2 * NK * 128, 1024], bf16)
    # host-pretiled: row p, col block t = original rows 128t+p
    wq0 = din("wq0", [128, HID], bf16)
    wq1 = din("wq1", [128, HID], bf16)
    wkv = din("wkv", [128, HID], bf16)     # [wv | wk] columns pretiled
    wof = din("wof", [128, NK * HID], bf16)  # FULL wo, pretiled
    cos2 = din("cos2", [128, S], bf16)
    ss2 = din("ss2", [128, S], bf16)
    ew_q = din("ew_q", [2, 128], bf16)
    ew_k = din("ew_k", [2, 128], bf16)
    e2 = din("e2", [2, 128], bf16)
    e2t = din("e2t", [128, 2], bf16)
    mask = din("mask", [128, 128], bf16)
    ident = din("ident", [64, 64], bf16)

    out_rs = nc.dram_tensor("out_rs", [256, S], f32,
                            kind="ExternalOutput").ap()

    with tile.TileContext(nc) as tc:
        with tc.tile_pool(name="consts", bufs=1) as cp, \
             tc.tile_pool(name="dram", bufs=1, space="DRAM") as dp:
            c_wq0 = cp.tile([128, HID], bf16, tag="w")
            c_wq1 = cp.tile([128, HID], bf16, tag="w2")
            c_wkv = cp.tile([128, HID], bf16, tag="w3")
            c_wo = cp.tile([128, NK * HID], bf16, tag="w4")
            c_cos = cp.tile([128, S], bf16, tag="c1")
            c_ss = cp.tile([128, S], bf16, tag="c2")
            c_ewq = cp.tile([2, 128], bf16, tag="c3")
            c_ewk = cp.tile([2, 128], bf16, tag="c4")
            c_e2 = cp.tile([2, 128], bf16, tag="c5")
            c_e2t = cp.tile([128, 2], bf16, tag="c5t")
            c_mask = cp.tile([128, 128], bf16, tag="c6")
            c_id = cp.tile([64, 64], bf16, tag="c7")
            c_eps = cp.tile([2, 1], f32, tag="c8")
            c_scr = cp.tile([128, 640], bf16, tag="c9")

            # PE warmup: memset scratch, then dummy matmuls so the HAM
            # clock gate is at 8/8 when the first real matmul lands.
            nc.vector.memset(c_scr[:], 0.0)
            nc.vector.memset(c_eps[:], RMS_EPS)

            # wq0 + even xT tiles on sync (gate the first matmuls);
            # wq1/wkv + odd xT tiles + small consts on scalar
            for h in range(4):
                cs_ = slice(512 * h, 512 * (h + 1))
                nc.sync.dma_start(c_wq0[:, cs_], wq0[:, cs_])
            nc.scalar.dma_start(c_wq1[:], wq1)
            nc.scalar.dma_start(c_wkv[:], wkv)
            nc.scalar.dma_start(c_e2t[:], e2t)
            nc.scalar.dma_start(c_ewq[:], ew_q)
            nc.scalar.dma_start(c_ewk[:], ew_k)
            nc.scalar.dma_start(c_id[:], ident)
            nc.scalar.dma_start(c_e2[:], e2)
            # rope tables + CDE consts + full wo on the gpsimd queue
            nc.gpsimd.dma_start(c_cos[:], cos2)
            nc.gpsimd.dma_start(c_ss[:], ss2)
            nc.gpsimd.dma_start(c_mask[:], mask)

            qkv = {
                "q0": cp.tile([128, S], bf16, tag="q0", name="q0"),
                "q1": cp.tile([128, S], bf16, tag="q1", name="q1"),
                "kv": cp.tile([128, S], bf16, tag="kv", name="kv"),
            }
            # per-half rope'd q/k and V_aug tiles
            qr0h = [cp.tile([128, 1024], bf16, tag=f"qr0{h}",
                            name=f"qr0{h}") for h in range(2)]
            qr1h = [cp.tile([128, 1024], bf16, tag=f"qr1{h}",
                            name=f"qr1{h}") for h in range(2)]
            krdh = [cp.tile([128, 1024], bf16, tag=f"krd{h}",
                            name=f"krd{h}") for h in range(2)]
            vah = [cp.tile([128, 8 * (HD + 1)], bf16, tag=f"va{h}",
                           name=f"va{h}") for h in range(2)]

            attn_bf = [cp.tile([128, S], bf16, tag=f"abf{i}",
                               name=f"abf{i}") for i in range(2)]
            l_sb = [cp.tile([2, S], bf16, tag=f"lsb{i}", name=f"lsb{i}")
                    for i in range(2)]

            a2a_in = [dp.tile([S, 128], bf16, name=f"a2ai{r}",
                              tag=f"ai{r}") for r in range(2)]
            a2a_out = [dp.tile([S, 128], bf16, name=f"a2ao{r}",
                               tag=f"ao{r}") for r in range(2)]

            # ================ phase A+B, halves ================
            with tc.tile_pool(name="xt", bufs=3) as xp, \
                 tc.tile_pool(name="sbB", bufs=2) as sbB:

                def phase_a(qh, psA, psM):
                    hs = slice(1024 * qh, 1024 * qh + 1024)
                    pq = [psA.tile([128, 1024], f32, tag="pa",
                                   name=f"pa{qh}_{j}") for j in range(3)]
                    for t in range(NK):
                        xt = xp.tile([128, 1024], bf16, tag="xt")
                        eng = (nc.sync, nc.scalar, nc.gpsimd)[t % 3]
                        xr = (qh * NK + t) * 128
                        eng.dma_start(xt[:], xP[xr:xr + 128, :])
                        st = (t == 0)
                        sp = (t == NK - 1)
                        tc_ = slice(128 * t, 128 * (t + 1))
                        for j, w in ((0, c_wq0), (1, c_wq1), (2, c_wkv)):
                            nc.tensor.matmul(pq[j][:, 0:512], w[:, tc_],
                                             xt[:, 0:512],
                                             start=st, stop=sp)
                            nc.tensor.matmul(pq[j][:, 512:1024],
                                             w[:, tc_], xt[:, 512:1024],
                                             start=st, stop=sp)
                    for j, key in ((0, "q0"), (1, "q1"), (2, "kv")):
                        nc.vector.tensor_copy(qkv[key][:, hs], pq[j][:])

                def phase_b_spec(qh, si, key, ew, dst, is_kv, psM,
                                 ptag="m"):
                    hs = slice(1024 * qh, 1024 * qh + 1024)
                    src = qkv[key]
                    if is_kv:
                        nc.gpsimd.memset(vah[qh][:], 1.0)
                        for lt in range(8):
                            ptr = psM.tile([128, 64], bf16, tag=ptag,
                                           name=f"ptr{qh}_{lt}")
                            nc.tensor.transpose(
                                ptr[:],
                                src[0:64, 1024 * qh + 128 * lt:
                                    1024 * qh + 128 * (lt + 1)],
                                c_id[:])
                            nc.vector.tensor_copy(
                                vah[qh][:, (HD + 1) * lt:
                                        (HD + 1) * lt + HD],
                                ptr[:])
                    sq = sbB.tile([128, 1024], bf16, tag="sq",
                                  bufs=2, name=f"sq{qh}_{si}")
                    nc.vector.tensor_mul(sq[:], src[:, hs], src[:, hs])
                    rstds = {}
                    for u in range(2):
                        us = slice(512 * u, 512 * u + 512)
                        pss = psM.tile([2, 512], f32, tag=ptag,
                                       name=f"ss{qh}_{si}_{u}")
                        nc.tensor.matmul(pss[:], c_e2t[:], sq[:, us],
                                         start=True, stop=True)
                        lnv = sbB.tile([2, 512], bf16, tag="lnv",
                                       bufs=4, name=f"lnv{qh}{si}{u}")
                        nc.scalar.activation(lnv[:], pss[:], AF.Ln,
                                             scale=1.0 / HD,
                                             bias=c_eps[:])
                        rr = sbB.tile([2, 512], bf16, tag="rstdr",
                                      bufs=4, name=f"rr{qh}{si}{u}")
                        nc.scalar.activation(rr[:], lnv[:],
                                             AF.Exp, scale=-0.5)
                        rstds[u] = rr
                    rows = slice(64, 128) if is_kv else slice(0, 128)
                    nrm = sbB.tile([128, 1024], f32, tag="nrm",
                                   bufs=2, name=f"nrm{qh}_{si}")
                    for u in range(2):
                        cs = slice(1024 * qh + 512 * u,
                                   1024 * qh + 512 * u + 512)
                        us = slice(512 * u, 512 * u + 512)
                        pb = psM.tile([128, 512], f32, tag=ptag,
                                      name=f"pb{qh}_{si}_{u}")
                        nc.tensor.matmul(pb[:], ew[:], rstds[u][:],
                                         start=True, stop=True)
                        nc.vector.tensor_mul(nrm[rows, us],
                                             src[rows, cs], pb[rows, :])
                    # rope
                    sh = sbB.tile([128, 1024], f32, tag="sh",
                                  bufs=2, name=f"sh{qh}_{si}")
                    if is_kv:
                        nc.sync.dma_start(sh[64:96, :], nrm[96:128, :])
                        nc.sync.dma_start(sh[96:128, :], nrm[64:96, :])
                    else:
                        nc.sync.dma_start(sh[0:32, :], nrm[32:64, :])
                        nc.sync.dma_start(sh[32:64, :], nrm[0:32, :])
                        nc.sync.dma_start(sh[64:96, :], nrm[96:128, :])
                        nc.sync.dma_start(sh[96:128, :], nrm[64:96, :])
                    t2 = sbB.tile([128, 1024], f32, tag="t2",
                                  bufs=1, name=f"t2{qh}_{si}")
                    nc.vector.tensor_mul(t2[rows, :], sh[rows, :],
                                         c_ss[rows, hs])
                    t1 = sbB.tile([128, 1024], f32, tag="sh",
                                  bufs=2, name=f"t1{qh}_{si}")
                    nc.vector.tensor_mul(t1[rows, :], nrm[rows, :],
                                         c_cos[rows, hs])
                    nc.vector.tensor_add(dst[rows, :], t1[rows, :],
                                         t2[rows, :])
                    if is_kv:
                        nc.sync.dma_start(dst[0:64, :], dst[64:128, :])

                # ---- scope 1: warmup + A0 + B0 + A1 ----
                with tc.tile_pool(name="psA", bufs=3,
                                  space="PSUM") as psA, \
                     tc.tile_pool(name="psM", bufs=2,
                                  space="PSUM") as psM:
                    pwm = psM.tile([128, 512], f32, tag="m", name="pwm")
                    for i in range(40):
                        nc.tensor.matmul(pwm[:], c_scr[:, 0:128],
                                         c_scr[:, 128:640],
                                         start=True, stop=True)
                    phase_a(0, psA, psM)
                    for si, (key, ew, dst, ik) in enumerate((
                            ("kv", c_ewk, krdh[0], True),
                            ("q0", c_ewq, qr0h[0], False),
                            ("q1", c_ewq, qr1h[0], False))):
                        phase_b_spec(0, si, key, ew, dst, ik, psM)
                    phase_a(1, psA, psM)
                    for h in range(8):
                        cs_ = slice(4096 * h, 4096 * (h + 1))
                        nc.gpsimd.dma_start(c_wo[:, cs_], wof[:, cs_])

                # ---- scope 2: B1 interleaved with CDE ----
                with tc.tile_pool(name="sbC", bufs=4) as sbC, \
                     tc.tile_pool(name="atk", bufs=2) as akp, \
                     tc.tile_pool(name="psS", bufs=2,
                                  space="PSUM") as psS, \
                     tc.tile_pool(name="psPV", bufs=2,
                                  space="PSUM") as psPV, \
                     tc.tile_pool(name="psO", bufs=2,
                                  space="PSUM") as psO:

                    def qchunk(qc):
                        qs = slice(512 * qc, 512 * qc + 512)
                        qhh = qc // 2
                        for hp, qrh in ((0, qr0h), (1, qr1h)):
                            qr = qrh[qhh]
                            ppv_a = psPV.tile([65, 512], f32, tag="pv")
                            ppv_b = psPV.tile([65, 512], f32, tag="pv")
                            ntile = 4 * qc + 4
                            for t in range(ntile):
                                r = t - 4 * qc
                                off = max(0, r) * 128
                                qlo = 512 * qc + off - 1024 * qhh
                                qlen = 512 * qc + 512 - 1024 * qhh - qlo
                                kh = t // 8
                                krd = krdh[kh]
                                v_aug = vah[kh]
                                tl = t - 8 * kh
                                kc = slice(128 * tl, 128 * (tl + 1))
                                vs = slice((HD + 1) * tl,
                                           (HD + 1) * tl + HD + 1)
                                st = (t == 0)
                                sp = (t == ntile - 1)
                                ps_s = psS.tile([128, 1024], f32,
                                                tag="s")
                                nc.tensor.matmul(
                                    ps_s[:, 0:qlen], krd[0:64, kc],
                                    qr[0:64, qlo:qlo + qlen],
                                    start=True, stop=True)
                                nc.tensor.matmul(
                                    ps_s[:, 512:512 + qlen],
                                    krd[64:128, kc],
                                    qr[64:128, qlo:qlo + qlen],
                                    start=True, stop=True)
                                pt = sbC.tile([128, 1024], bf16,
                                              tag="pt")
                                if r >= 0:
                                    nc.scalar.activation(
                                        pt[:, 0:512 + qlen],
                                        ps_s[:, 0:512 + qlen],
                                        AF.Exp, scale=SCALING)
                                    nc.vector.tensor_mul(
                                        pt[:, 0:128], pt[:, 0:128],
                                        c_mask[:])
                                    nc.vector.tensor_mul(
                                        pt[:, 512:640], pt[:, 512:640],
                                        c_mask[:])
                                else:
                                    nc.scalar.activation(
                                        pt[:, 0:1024], ps_s[:, 0:1024],
                                        AF.Exp, scale=SCALING)
                                nc.tensor.matmul(
                                    ppv_a[:, off:512], v_aug[:, vs],
                                    pt[:, 0:qlen], start=st, stop=sp)
                                nc.tensor.matmul(
                                    ppv_b[:, off:512], v_aug[:, vs],
                                    pt[:, 512:512 + qlen],
                                    start=st, stop=sp)
                            # per-hp: stage, 1/l, normalize, a2a-stage
                            for half, ppv in ((0, ppv_a), (1, ppv_b)):
                                stg = sbC.tile([65, 512], bf16,
                                               tag="stg", bufs=3)
                                nc.vector.tensor_copy(stg[:], ppv[:])
                                nc.sync.dma_start(
                                    attn_bf[hp][64 * half:
                                                64 * half + 64, qs],
                                    stg[0:64, :])
                                nc.sync.dma_start(
                                    l_sb[hp][half:half + 1, qs],
                                    stg[64:65, :])
                            rl = sbC.tile([2, 512], f32, tag="lnl",
                                          bufs=2, name=f"rl{qc}_{hp}")
                            nc.vector.reciprocal(rl[:],
                                                 l_sb[hp][:, qs])
                            rlb = sbC.tile([2, 512], bf16, tag="rlb",
                                           bufs=2, name=f"rb{qc}_{hp}")
                            nc.vector.tensor_copy(rlb[:], rl[:])
                            pb = psO.tile([128, 512], f32, tag="o")
                            nc.tensor.matmul(pb[:], c_e2[:], rlb[:],
                                             start=True, stop=True)
                            nc.vector.tensor_mul(
                                attn_bf[hp][:, qs],
                                attn_bf[hp][:, qs], pb[:])
                            rnd = qc // 2
                            seng = nc.gpsimd if hp == 0 else nc.sync
                            for j in range(4 * (qc % 2),
                                           4 * (qc % 2) + 4):
                                qq = 1024 * rnd + 128 * j
                                rr_ = 256 * j + 128 * hp
                                seng.dma_start(
                                    a2a_in[rnd][rr_:rr_ + 128, :],
                                    attn_bf[hp][:, qq:qq + 128])
                        if qc % 2 == 1:
                            rnd = qc // 2
                            nc.gpsimd.collective_compute(
                                "AllToAll",
                                mybir.AluOpType.bypass,
                                replica_groups=[list(range(N_CORES))],
                                ins=[a2a_in[rnd][:, :].opt()],
                                outs=[a2a_out[rnd][:, :].opt()],
                            )

                    def oproj(rnd):
                        attk = akp.tile([128, S], bf16, tag="atk")
                        for kk in range(NK):
                            eng = nc.gpsimd if kk % 2 == 0 else \
                                nc.sync
                            eng.dma_start(
                                attk[:, 128 * kk:128 * (kk + 1)],
                                a2a_out[rnd][128 * kk:
                                             128 * (kk + 1), :])
                        pos = [psO.tile([128, 512], f32, tag="o",
                                        name=f"po{rnd}_0"),
                               psO.tile([128, 512], f32, tag="o",
                                        name=f"po{rnd}_1"),
                               psS.tile([128, 512], f32, tag="s",
                                        name=f"po{rnd}_2"),
                               psS.tile([128, 512], f32, tag="s",
                                        name=f"po{rnd}_3")]
                        for kk in range(NK):
                            for n in range(4):
                                nc.tensor.matmul(
                                    pos[n][:],
                                    attk[:, 128 * kk:128 * (kk + 1)],
                                    c_wo[:, HID * kk + 512 * n:
                                         HID * kk + 512 * n + 512],
                                    start=(kk == 0),
                                    stop=(kk == NK - 1))
                        for n in range(4):
                            ost = sbC.tile([128, 512], f32, tag="ost",
                                           bufs=2)
                            nc.vector.tensor_copy(ost[:], pos[n][:])
                            nc.sync.dma_start(
                                out_rs[128 * rnd:128 * rnd + 128,
                                       512 * n:512 * n + 512],
                                ost[:])

                    phase_b_spec(1, 0, "kv", c_ewk, krdh[1], True,
                                 psO, ptag="o")
                    qchunk(0)
                    phase_b_spec(1, 1, "q0", c_ewq, qr0h[1], False,
                                 psO, ptag="o")
                    qchunk(1)
                    phase_b_spec(1, 2, "q1", c_ewq, qr1h[1], False,
                                 psO, ptag="o")
                    qchunk(2)
                    qchunk(3)
                    oproj(0)
                    oproj(1)

    nc.compile()
    return nc


def _host_prep(hidden_states, position_ids, wq, wk, wv, wo, q_ln_w, k_ln_w):
    x = np.asarray(hidden_states, dtype=np.float32)[0]        # [S, HID]
    xT = np.ascontiguousarray(x.T).astype(BF16NP)             # [HID, S]
    # [half, ktile, 128, 1024]: per-(half, ktile) contiguous 256 KB
    xP = np.ascontiguousarray(
        xT.reshape(NK, 128, 2, 1024).transpose(2, 0, 1, 3)
    ).reshape(2 * NK * 128, 1024)
    pos = np.asarray(position_ids)[0].astype(np.float32)      # [S]
    inv = 1.0 / (ROPE_THETA ** (np.arange(0, HD, 2, dtype=np.float32) / HD))
    ang = pos[:, None] * inv[None, :]                         # [S, 32]
    emb = np.concatenate([ang, ang], axis=1)                  # [S, 64]
    cosT = np.cos(emb).T.astype(np.float32)                   # [64, S]
    sinT = np.sin(emb).T.astype(np.float32)
    ss = sinT.copy()
    ss[0:32] = -sinT[0:32]
    cos2 = np.tile(cosT, (2, 1))
    ss2 = np.tile(ss, (2, 1))

    e2 = np.zeros((2, 128), dtype=np.float32)
    e2[0, 0:64] = 1.0
    e2[1, 64:128] = 1.0
    ew_q = np.zeros((2, 128), dtype=np.float32)
    ew_q[0, 0:64] = q_ln_w
    ew_q[1, 64:128] = q_ln_w
    ew_k = np.zeros((2, 128), dtype=np.float32)
    ew_k[1, 64:128] = k_ln_w
    msk = (np.arange(128)[:, None] <= np.arange(128)[None, :]) \
        .astype(np.float32)
    ident = np.eye(64, dtype=np.float32)

    wq_ = np.asarray(wq, dtype=np.float32)
    wk_ = np.asarray(wk, dtype=np.float32)
    wv_ = np.asarray(wv, dtype=np.float32)
    wo_ = np.asarray(wo, dtype=np.float32)

    def pretile(w):  # [HID, N] -> [128, NK*N] ktile-blocked
        n = w.shape[1]
        return np.ascontiguousarray(
            w.reshape(NK, 128, n).transpose(1, 0, 2).reshape(128, NK * n))

    wof = pretile(wo_).astype(BF16NP)

    in_maps = []
    for c in range(N_CORES):
        qcols = slice(256 * c, 256 * (c + 1))
        kvcols = slice(64 * c, 64 * (c + 1))
        wq_c = np.ascontiguousarray(wq_[:, qcols])
        wkv_c = np.concatenate([wv_[:, kvcols], wk_[:, kvcols]], axis=1)
        in_maps.append({
            "xP": xP,
            "wq0": pretile(wq_c[:, 0:128]).astype(BF16NP),
            "wq1": pretile(wq_c[:, 128:256]).astype(BF16NP),
            "wkv": pretile(wkv_c).astype(BF16NP),
            "wof": wof,
            "cos2": cos2.astype(BF16NP),
            "ss2": ss2.astype(BF16NP),
            "ew_q": ew_q.astype(BF16NP),
            "ew_k": ew_k.astype(BF16NP),
            "e2": e2.astype(BF16NP),
            "e2t": np.ascontiguousarray(e2.T).astype(BF16NP),
            "mask": msk.astype(BF16NP),
            "ident": ident.astype(BF16NP),
        })
    return in_maps


def kernel(hidden_states, position_ids, wq, wk, wv, wo, q_ln_w, k_ln_w):
    global _NC_CACHE, LAST_RESULTS
    if _NC_CACHE is None:
        _NC_CACHE = _build()
    nc = _NC_CACHE
    in_maps = _host_prep(hidden_states, position_ids, wq, wk, wv, wo,
                         q_ln_w, k_ln_w)
    res = bass_utils.run_bass_kernel_spmd(
        nc, in_maps, core_ids=list(range(N_CORES)))
    LAST_RESULTS = res
    out = np.empty((S, HID), dtype=np.float32)
    for c in range(N_CORES):
        o_c = res.results[c]["out_rs"]        # [256, 2048]
        for rnd in range(2):
            out[1024 * rnd + 128 * c:1024 * rnd + 128 * c + 128, :] = \
                o_c[128 * rnd:128 * rnd + 128, :]
    return out.reshape(1, S, HID)
